# revision 1
# baseline (speedup 1.0000x reference)
"""2-layer multi-head GAT on 8 Trainium2 NeuronCores (Bass/Tile, single launch).

Contract: kernel(**inputs) takes the FULL unsharded inputs (as produced by
setup_inputs()) and returns the FULL [N, 16] float32 output.

Distribution strategy (dst-sharded message passing):
  - Destination nodes are sharded across the 8 cores in 128-aligned shards;
    each core owns all edges whose dst falls in its shard.
  - phase 1 (per core): z1 = x @ [W1 | W1@A_src | W1@A_dst] for the core's
    own node shard -> 512B rows [z(64) | asrc(8) | adst(8) | pad] in HBM.
  - AllGather the z-row table so every core can gather any source row.
  - phase 2 (layer-1 edge phase): per 128-dst-node block, per-edge
    dma_gather (SWDGE, 4 queues) of source rows; softmax restructured as
    num/denom with exp(leaky(.)) per edge (no segment max needed - value
    range is small); per-edge dst-side stats are expanded node->edge with a
    node-major one-hot matmul (bf16 hi/lo exact split); messages scaled by
    exp weights and segment-summed dst-wise with an edge-major one-hot
    matmul accumulating in PSUM. Epilogue: divide by denom, +b1, ELU ->
    H rows + layer-2 per-node stats.
  - AllGather H table; phase 4 repeats the edge phase with 1 head and
    finishes with out2 = (num/den) @ W2 + b2, written transposed.
Small weights are replicated to all cores. Edge index preprocessing
(partitioning, sorting, padding, CSR run bounds) happens on host.
"""
import math
from contextlib import ExitStack

import numpy as np

import concourse.bass as bass
import concourse.bacc as bacc
import concourse.mybir as mybir
import concourse.tile as tile
import concourse.bass2jax as b2j
from concourse.library_config import mlp
from concourse.tile_rust import add_dep_helper

F32 = mybir.dt.float32
BF16 = mybir.dt.bfloat16
I16 = mybir.dt.int16
I32 = mybir.dt.int32
OP = mybir.AluOpType
AF = mybir.ActivationFunctionType

LEAKY = 0.2
PAD_DST = 200.0
N_CORES = 8


def cdiv(a, b):
    return -(-a // b)


def _wrap_idx16(idx):
    n = idx.shape[0]
    w = idx.reshape(n // 16, 16).T.astype(np.int16)
    return np.tile(w, (8, 1))


def host_prep(x, edge_indices, W1, a_src1, a_dst1, b1, W2, a_src2, a_dst2, b2,
              n_cores=N_CORES):
    N, Din = x.shape
    D1 = W1.shape[1]
    H1, Dh1 = a_src1.shape
    D2 = W2.shape[1]
    assert Din == 128 and D1 == 64 and H1 * Dh1 == D1

    SHARD = cdiv(N, n_cores * 128) * 128
    NTOT = SHARD * n_cores
    HALF = NTOT // 2
    B = SHARD // 128
    assert HALF % 128 == 0 and HALF <= 32768

    A_src = np.zeros((D1, H1), np.float64)
    A_dst = np.zeros((D1, H1), np.float64)
    for h in range(H1):
        A_src[h * Dh1:(h + 1) * Dh1, h] = a_src1[h]
        A_dst[h * Dh1:(h + 1) * Dh1, h] = a_dst1[h]
    W1f = np.concatenate([W1.astype(np.float64),
                          W1.astype(np.float64) @ A_src,
                          W1.astype(np.float64) @ A_dst], axis=1).astype(np.float32)
    w_as2 = (W2.astype(np.float64) @ a_src2[0].astype(np.float64)).astype(np.float32)
    w_ad2 = (W2.astype(np.float64) @ a_dst2[0].astype(np.float64)).astype(np.float32)

    xT = np.zeros((128, NTOT), np.float32)
    xT[:, :N] = x.T

    src = np.asarray(edge_indices[0], np.int64)
    dst = np.asarray(edge_indices[1], np.int64)
    core = dst // SHARD

    percore = []
    nlo = np.zeros((n_cores, B), np.int64)
    nhi = np.zeros((n_cores, B), np.int64)
    for c in range(n_cores):
        m = core == c
        s_c, d_c = src[m], dst[m]
        blk = (d_c - c * SHARD) >> 7
        half = (s_c >= HALF).astype(np.int64)
        order = np.lexsort((d_c, half, blk))
        s_c, d_c, blk, half = s_c[order], d_c[order], blk[order], half[order]
        cnt = np.bincount(blk * 2 + half, minlength=2 * B)
        nlo[c] = cnt[0::2]
        nhi[c] = cnt[1::2]
        percore.append((s_c, d_c, blk, half))

    NLO = np.maximum(cdiv(nlo, 128).max(axis=0), 1)
    NHI = np.maximum(cdiv(nhi, 128).max(axis=0), 1)
    NSUB = int((NLO + NHI).sum())
    TOT = NSUB * 128

    sub_off_lo = np.zeros(B, np.int64)
    sub_off_hi = np.zeros(B, np.int64)
    acc = 0
    for b in range(B):
        sub_off_lo[b] = acc
        acc += NLO[b]
        sub_off_hi[b] = acc
        acc += NHI[b]

    gops = []
    for b in range(B):
        ops = []
        for tbl, off, k in ((0, sub_off_lo[b], NLO[b]), (1, sub_off_hi[b], NHI[b])):
            done = 0
            while done < k:
                take = min(8, k - done)
                ops.append((tbl, int(off + done), int(take)))
                done += take
        gops.append(ops)

    streams = []
    for c in range(n_cores):
        s_c, d_c, blk, half = percore[c]
        idxs = np.zeros(TOT, np.int64)
        dstloc = np.full(TOT, PAD_DST, np.float32)
        pos = np.zeros(len(s_c), np.int64)
        for b in range(B):
            for hv, off_sub, _n in ((0, sub_off_lo[b], nlo[c, b]),
                                    (1, sub_off_hi[b], nhi[c, b])):
                m = (blk == b) & (half == hv)
                k = int(m.sum())
                if k:
                    pos[m] = off_sub * 128 + np.arange(k)
        idxs[pos] = np.where(half == 0, s_c, s_c - HALF)
        dstloc[pos] = (d_c - c * SHARD - blk * 128).astype(np.float32)

        subid = pos >> 7
        dl = dstloc[pos].astype(np.int64)
        cnt2 = np.bincount(subid * 128 + dl, minlength=NSUB * 128)
        cnt2 = cnt2.reshape(NSUB, 128).T.astype(np.float32)
        rp_lo = np.cumsum(cnt2, axis=0) - cnt2
        rp_hi = rp_lo + cnt2

        streams.append({
            "idx16": _wrap_idx16(idxs.astype(np.int16)),
            "dstloc": dstloc.reshape(NSUB, 128).T.copy(),
            "rplo": rp_lo, "rphi": rp_hi,
        })

    cfg = dict(N=N, n_cores=n_cores, SHARD=SHARD, NTOT=NTOT, HALF=HALF, B=B,
               NLO=NLO, NHI=NHI, NSUB=NSUB, TOT=TOT, gops=gops,
               H1=H1, Dh1=Dh1, D2=D2)
    consts = dict(
        W1f=W1f,
        b1r=np.tile(b1[None, :], (128, 1)).astype(np.float32),
        wa2r=np.tile(w_as2[None, :], (128, 1)).astype(np.float32),
        wd2r=np.tile(w_ad2[None, :], (128, 1)).astype(np.float32),
        W2=W2.astype(np.float32),
        b2c=b2.reshape(D2, 1).astype(np.float32),
    )
    in_maps = []
    for c in range(n_cores):
        m = dict(consts)
        m["xTs"] = np.ascontiguousarray(xT[:, c * SHARD:(c + 1) * SHARD])
        m.update(streams[c])
        in_maps.append(m)
    return cfg, in_maps


def build_nc(cfg, repeat=1):
    n_cores = cfg["n_cores"]
    SHARD, NTOT, HALF, B = cfg["SHARD"], cfg["NTOT"], cfg["HALF"], cfg["B"]
    NSUB, TOT = cfg["NSUB"], cfg["TOT"]
    gops = cfg["gops"]
    D2 = cfg["D2"]
    NLO, NHI = cfg["NLO"], cfg["NHI"]

    nc = bacc.Bacc("TRN2", target_bir_lowering=False, debug=False,
                   num_devices=n_cores, num_swdge_queues=4)

    din = {}
    for name, shape, dt in [
            ("xTs", [128, SHARD], F32), ("W1f", [128, 80], F32),
            ("b1r", [128, 64], F32), ("wa2r", [128, 64], F32),
            ("wd2r", [128, 64], F32), ("W2", [64, D2], F32), ("b2c", [D2, 1], F32),
            ("idx16", [128, TOT // 16], I16), ("dstloc", [128, NSUB], F32),
            ("rplo", [128, NSUB], F32), ("rphi", [128, NSUB], F32)]:
        din[name] = nc.dram_tensor(name, shape, dt, kind="ExternalInput").ap()

    z1shard = nc.dram_tensor("z1shard", [SHARD, 128], F32).ap()
    z1full = nc.dram_tensor("z1full", [NTOT, 128], F32).ap()
    h2shard = nc.dram_tensor("h2shard", [SHARD, 128], F32).ap()
    h2full = nc.dram_tensor("h2full", [NTOT, 128], F32).ap()
    out2T = nc.dram_tensor("out2T", [D2, SHARD], F32, kind="ExternalOutput").ap()

    with tile.TileContext(nc) as tc, ExitStack() as top:
        nc.gpsimd.load_library(mlp)
        cp = top.enter_context(tc.tile_pool(name="consts", bufs=1))

        xts = cp.tile([128, SHARD], F32)
        w1f = cp.tile([128, 80], F32)
        b1r = cp.tile([128, 64], F32)
        wa2r = cp.tile([128, 64], F32)
        wd2r = cp.tile([128, 64], F32)
        w2 = cp.tile([64, D2], F32)
        b2c = cp.tile([D2, 1], F32)
        idxs = cp.tile([128, TOT // 16], I16)
        dstl = cp.tile([128, NSUB], F32)
        rplo = cp.tile([128, NSUB], F32)
        rphi = cp.tile([128, NSUB], F32)
        for t, name in [(xts, "xTs"), (w1f, "W1f"), (b1r, "b1r"), (wa2r, "wa2r"),
                        (wd2r, "wd2r"), (w2, "W2"), (b2c, "b2c"), (idxs, "idx16"),
                        (dstl, "dstloc"), (rplo, "rplo"), (rphi, "rphi")]:
            nc.sync.dma_start(t[:], din[name][:])

        iota_i = cp.tile([128, 128], I32)
        iota_c = cp.tile([128, 1], I32)
        iota = cp.tile([128, 128], F32)
        iotac = cp.tile([128, 1], F32)
        ident = cp.tile([128, 128], F32)
        nc.gpsimd.iota(iota_i[:], [[1, 128]], base=0, channel_multiplier=0)
        nc.gpsimd.iota(iota_c[:], [[1, 1]], base=0, channel_multiplier=1)
        nc.vector.tensor_copy(iota[:], iota_i[:])
        nc.vector.tensor_copy(iotac[:], iota_c[:])
        nc.vector.tensor_scalar(ident[:], iota[:], iotac[:, :1], None,
                                op0=OP.is_equal)

        adst1p = cp.tile([128, B, 16], BF16)
        adst2p = cp.tile([128, B, 2], BF16)

        for _rep in range(repeat):
            # ---------- phase 1: z-prep ----------
            with tc.tile_pool(name="p1", bufs=3) as p1, \
                 tc.tile_pool(name="p1ps", bufs=2, space="PSUM") as p1ps:
                for t in range(B):
                    pz = p1ps.tile([128, 80], F32, space="PSUM")
                    nc.tensor.matmul(pz[:], xts[:, t * 128:(t + 1) * 128], w1f[:],
                                     start=True, stop=True)
                    zw = p1.tile([128, 128], F32)
                    nc.vector.tensor_copy(zw[:, 0:80], pz[:])
                    hif = p1.tile([128, 8], F32)
                    nc.vector.tensor_copy(adst1p[:, t, 0:8], zw[:, 72:80])
                    nc.vector.tensor_copy(hif[:], adst1p[:, t, 0:8])
                    nc.vector.tensor_tensor(adst1p[:, t, 8:16], zw[:, 72:80],
                                            hif[:], op=OP.subtract)
                    nc.sync.dma_start(z1shard[t * 128:(t + 1) * 128, :], zw[:])

            ag1 = nc.gpsimd.collective_compute(
                "AllGather", OP.bypass, replica_groups=[list(range(n_cores))],
                ins=[z1shard[:]], outs=[z1full[:]])

            def edge_phase(table, adstp, nst, nh, out_cb, ag_inst):
                with tc.tile_pool(name="zg", bufs=4) as zgp, \
                     tc.tile_pool(name="stag", bufs=4) as stp, \
                     tc.tile_pool(name="oh", bufs=3) as ohp, \
                     tc.tile_pool(name="sm", bufs=3) as smp, \
                     tc.tile_pool(name="ee", bufs=4) as eep, \
                     tc.tile_pool(name="pm", bufs=2, space="PSUM") as pmp, \
                     tc.tile_pool(name="pa", bufs=2, space="PSUM") as pap, \
                     tc.tile_pool(name="epi", bufs=3) as epi, \
                     tc.tile_pool(name="eps", bufs=1, space="PSUM") as epips:
                    qn = 0
                    for b in range(B):
                        pmain = pmp.tile([128, nst], F32, space="PSUM")
                        nsub_b = int(NLO[b] + NHI[b])
                        si = 0
                        for (tbl, sub0, k) in gops[b]:
                            zg = zgp.tile([128, 8, 128], F32, tag="zg")
                            tab = (table[0:HALF, :] if tbl == 0
                                   else table[HALF:NTOT, :])
                            g = nc.gpsimd.dma_gather(
                                zg[:, 0:k, :], tab,
                                idxs[:, sub0 * 8:(sub0 + k) * 8],
                                k * 128, k * 128, 128,
                                single_packet=True, queue_num=qn % 4)
                            if ag_inst is not None:
                                add_dep_helper(g.ins, ag_inst.ins, sync=True,
                                               reason="gather after allgather")
                            qn += 1
                            st = ohp.tile([128, 8, 128], F32, tag="st")
                            iob = bass.AP(iota.tensor, iota[:].offset,
                                          [[iota[:].ap[0][0], 128], [0, k],
                                           [1, 128]])
                            dlb = bass.AP(dstl.tensor, dstl[:].offset + sub0,
                                          [[dstl[:].ap[0][0], 128], [1, k],
                                           [0, 128]])
                            nc.vector.tensor_tensor(st[:, 0:k, :], iob, dlb,
                                                    op=OP.is_equal)
                            ge = smp.tile([128, 8, 128], F32, tag="ge")
                            sn = smp.tile([128, 8, 128], BF16, tag="sn")
                            rlb = bass.AP(rplo.tensor, rplo[:].offset + sub0,
                                          [[rplo[:].ap[0][0], 128], [1, k],
                                           [0, 128]])
                            rhb = bass.AP(rphi.tensor, rphi[:].offset + sub0,
                                          [[rphi[:].ap[0][0], 128], [1, k],
                                           [0, 128]])
                            nc.vector.tensor_tensor(ge[:, 0:k, :], iob, rlb,
                                                    op=OP.is_ge)
                            lt = ohp.tile([128, 8, 128], F32, tag="lt")
                            nc.vector.tensor_tensor(lt[:, 0:k, :], iob, rhb,
                                                    op=OP.is_lt)
                            nc.vector.tensor_tensor(sn[:, 0:k, :], ge[:, 0:k, :],
                                                    lt[:, 0:k, :], op=OP.mult)
                            pa = pap.tile([128, 8, 2 * nh], F32, space="PSUM",
                                          tag="pa")
                            stag = stp.tile([128, 8, nst], F32, tag="stag")
                            for s in range(k):
                                nc.tensor.matmul(pa[:, s, :], sn[:, s, :],
                                                 adstp[:, b, :], start=True,
                                                 stop=True)
                            et = eep.tile([128, 8, nh], F32, tag="et")
                            ev = eep.tile([128, 8, nh], F32, tag="ev")
                            nc.vector.tensor_tensor(
                                et[:, 0:k, :], zg[:, 0:k, 64:64 + nh],
                                pa[:, 0:k, 0:nh], op=OP.add)
                            nc.vector.tensor_tensor(
                                ev[:, 0:k, :], et[:, 0:k, :],
                                pa[:, 0:k, nh:2 * nh], op=OP.add)
                            lr = eep.tile([128, 8, nh], F32, tag="lr")
                            nc.scalar.activation(lr[:, 0:k, :], ev[:, 0:k, :],
                                                 AF.Prelu, alpha=LEAKY)
                            exp_out = bass.AP(
                                stag.tensor, stag[:].offset + 64,
                                [[stag[:].ap[0][0], 128], [nst, k], [1, nh]])
                            nc.scalar.activation(exp_out, lr[:, 0:k, :], AF.Exp)
                            expb = bass.AP(
                                stag.tensor, stag[:].offset + 64,
                                [[stag[:].ap[0][0], 128], [nst, k], [1, nh],
                                 [0, 64 // nh]])
                            nc.vector.tensor_tensor(stag[:, 0:k, 0:64],
                                                    zg[:, 0:k, 0:64], expb,
                                                    op=OP.mult)
                            for s in range(k):
                                nc.tensor.matmul(pmain[:], st[:, s, :],
                                                 stag[:, s, 0:nst],
                                                 start=(si + s == 0),
                                                 stop=(si + s == nsub_b - 1))
                            si += k
                        out_cb(b, pmain, epi, epips)

            h2w_holder = []

            def epi1(b, pmain, epi, epips):
                den = epi.tile([128, 8], F32, tag="den")
                nc.vector.tensor_scalar(den[:], pmain[:, 64:72], 1e-16, None,
                                        op0=OP.add)
                rden = epi.tile([128, 8], F32, tag="rden")
                nc.vector.reciprocal(rden[:], den[:])
                o1 = epi.tile([128, 64], F32, tag="o1")
                rdb = bass.AP(rden.tensor, rden[:].offset,
                              [[rden[:].ap[0][0], 128], [1, 8], [0, 8]])
                nc.vector.tensor_tensor(o1[:], pmain[:, 0:64], rdb, op=OP.mult)
                x1 = epi.tile([128, 64], F32, tag="x1")
                nc.vector.tensor_tensor(x1[:], o1[:], b1r[:], op=OP.add)
                xm = epi.tile([128, 64], F32, tag="xm")
                nc.vector.tensor_scalar(xm[:], x1[:], 0.0, None, op0=OP.min)
                u = epi.tile([128, 64], F32, tag="u")
                nc.scalar.activation(u[:], xm[:], AF.Exp)
                v = epi.tile([128, 64], F32, tag="v")
                nc.vector.tensor_scalar(v[:], x1[:], 0.0, None, op0=OP.max)
                hw = epi.tile([128, 128], F32, tag="hw")
                nc.vector.scalar_tensor_tensor(hw[:, 0:64], u[:], -1.0, v[:],
                                               op0=OP.add, op1=OP.add)
                tr = epi.tile([128, 64], F32, tag="tr")
                tr2 = epi.tile([128, 64], F32, tag="tr2")
                nc.vector.scalar_tensor_tensor(tr[:], hw[:, 0:64], 1.0, wa2r[:],
                                               op0=OP.mult, op1=OP.mult,
                                               accum_out=hw[:, 64:65])
                t2 = epi.tile([128, 1], F32, tag="t2")
                nc.vector.scalar_tensor_tensor(tr2[:], hw[:, 0:64], 1.0, wd2r[:],
                                               op0=OP.mult, op1=OP.mult,
                                               accum_out=t2[:])
                hif = epi.tile([128, 1], F32, tag="hif")
                nc.vector.tensor_copy(adst2p[:, b, 0:1], t2[:])
                nc.vector.tensor_copy(hif[:], adst2p[:, b, 0:1])
                nc.vector.tensor_tensor(adst2p[:, b, 1:2], t2[:], hif[:],
                                        op=OP.subtract)
                w = nc.sync.dma_start(h2shard[b * 128:(b + 1) * 128, :], hw[:])
                h2w_holder.append(w)

            edge_phase(z1full, adst1p, 72, 8, epi1, ag1)

            ag2 = nc.gpsimd.collective_compute(
                "AllGather", OP.bypass, replica_groups=[list(range(n_cores))],
                ins=[h2shard[:]], outs=[h2full[:]])
            for w in h2w_holder:
                add_dep_helper(ag2.ins, w.ins, sync=True,
                               reason="h2 write before ag2")

            def epi2(b, pmain, epi, epips):
                den = epi.tile([128, 1], F32, tag="den")
                nc.vector.tensor_scalar(den[:], pmain[:, 64:65], 1e-16, None,
                                        op0=OP.add)
                rden = epi.tile([128, 1], F32, tag="rden")
                nc.vector.reciprocal(rden[:], den[:])
                agg = epi.tile([128, 64], F32, tag="agg")
                nc.vector.tensor_scalar(agg[:], pmain[:, 0:64], rden[:, :1],
                                        None, op0=OP.mult)
                ptr = epips.tile([64, 128], F32, space="PSUM", tag="ptr")
                nc.tensor.transpose(ptr[:], agg[:], ident[:])
                aggT = epi.tile([64, 128], F32, tag="aggT")
                nc.vector.tensor_copy(aggT[:], ptr[:])
                po2 = epips.tile([D2, 128], F32, space="PSUM", tag="po2")
                nc.tensor.matmul(po2[:], w2[:], aggT[:], start=True, stop=True)
                o2 = epi.tile([D2, 128], F32, tag="o2")
                nc.scalar.activation(o2[:], po2[:], AF.Identity, bias=b2c[:, :1])
                nc.sync.dma_start(out2T[:, b * 128:(b + 1) * 128], o2[:])

            edge_phase(h2full, adst2p, 65, 1, epi2, ag2)

    nc.compile()
    return nc


class CachedRunner:
    def __init__(self, nc, n_cores):
        import jax
        from jax.sharding import Mesh, PartitionSpec, NamedSharding
        from jax.experimental.shard_map import shard_map
        b2j.install_neuronx_cc_hook()
        self.nc = nc
        self.n_cores = n_cores
        in_names, out_names, out_avals = [], [], []
        for alloc in nc.m.functions[0].allocations:
            if not isinstance(alloc, mybir.MemoryLocationSet):
                continue
            name = alloc.memorylocations[0].name
            if alloc.kind == "ExternalInput":
                if (nc.partition_id_tensor is None
                        or name != nc.partition_id_tensor.name):
                    in_names.append(name)
            elif alloc.kind == "ExternalOutput":
                out_names.append(name)
                out_avals.append(jax.core.ShapedArray(
                    tuple(alloc.tensor_shape), mybir.dt.np(alloc.dtype)))
        self.in_names, self.out_names, self.out_avals = \
            in_names, out_names, out_avals
        n_params = len(in_names)
        all_in = list(in_names) + list(out_names)
        if nc.partition_id_tensor is not None:
            all_in.append(nc.partition_id_tensor.name)

        def _body(*args):
            operands = list(args)
            if nc.partition_id_tensor is not None:
                operands.append(b2j.partition_id_tensor())
            outs = b2j._bass_exec_p.bind(
                *operands, out_avals=tuple(out_avals), in_names=tuple(all_in),
                out_names=tuple(out_names), lowering_input_output_aliases=(),
                sim_require_finite=True, sim_require_nnan=True, nc=nc)
            return tuple(outs)

        self.jax = jax
        self.devices = jax.devices()[:n_cores]
        self.mesh = Mesh(np.asarray(self.devices), ("core",))
        donate = tuple(range(n_params, n_params + len(out_names)))
        self.fn = jax.jit(
            shard_map(_body, mesh=self.mesh,
                      in_specs=(PartitionSpec("core"),) * (n_params +
                                                           len(out_names)),
                      out_specs=(PartitionSpec("core"),) * len(out_names),
                      check_rep=False),
            donate_argnums=donate, keep_unused=True)
        self.sh = NamedSharding(self.mesh, PartitionSpec("core"))
        self.dev_ins = None

    def put_inputs(self, in_maps):
        concat = [np.concatenate([np.asarray(in_maps[c][n])
                                  for c in range(self.n_cores)], axis=0)
                  for n in self.in_names]
        self.dev_ins = [self.jax.device_put(a, self.sh) for a in concat]
        for a in self.dev_ins:
            a.block_until_ready()

    def __call__(self):
        jnp = self.jax.numpy
        zeros = [self.jax.device_put(
            jnp.zeros((self.n_cores * av.shape[0], *av.shape[1:]), av.dtype),
            self.sh) for av in self.out_avals]
        outs = self.fn(*self.dev_ins, *zeros)
        return {name: np.asarray(outs[i]).reshape(
                    self.n_cores, *self.out_avals[i].shape)
                for i, name in enumerate(self.out_names)}


_STATE = {}


def _fingerprint(inputs):
    import hashlib
    h = hashlib.sha256()
    for k in sorted(inputs):
        a = np.asarray(inputs[k])
        h.update(k.encode())
        h.update(str(a.shape).encode())
        h.update(str(a.dtype).encode())
        h.update(np.ascontiguousarray(a).tobytes())
    return h.hexdigest()


def _get_state(inputs, repeat=1):
    key = (_fingerprint(inputs), repeat)
    st = _STATE.get("st")
    if st is not None and st["key"] == key:
        return st
    cfg, in_maps = host_prep(
        np.asarray(inputs["x"], np.float32),
        np.asarray(inputs["edge_indices"]),
        np.asarray(inputs["W1"], np.float32),
        np.asarray(inputs["a_src1"], np.float32),
        np.asarray(inputs["a_dst1"], np.float32),
        np.asarray(inputs["b1"], np.float32),
        np.asarray(inputs["W2"], np.float32),
        np.asarray(inputs["a_src2"], np.float32),
        np.asarray(inputs["a_dst2"], np.float32),
        np.asarray(inputs["b2"], np.float32))
    nc = build_nc(cfg, repeat=repeat)
    runner = CachedRunner(nc, cfg["n_cores"])
    runner.put_inputs(in_maps)
    st = {"key": key, "cfg": cfg, "runner": runner}
    _STATE["st"] = st
    return st


def kernel(**inputs):
    st = _get_state(inputs)
    res = st["runner"]()
    cfg = st["cfg"]
    out = np.concatenate([res["out2T"][c].T for c in range(cfg["n_cores"])],
                         axis=0)[:cfg["N"]]
    return np.ascontiguousarray(out.astype(np.float32))


def measure_device_time_ns(inputs, k0=1, k1=5, reps=10):
    """Slope-based device-time estimate: build K-repeat variants of the whole
    kernel body and difference the best wall times (fixed dispatch cancels)."""
    import time
    times = {}
    saved = _STATE.pop("st", None)
    for K in (k0, k1):
        st = _get_state(inputs, repeat=K)
        st["runner"]()
        st["runner"]()
        best = 1e9
        for _ in range(reps):
            t0 = time.time()
            st["runner"]()
            best = min(best, time.time() - t0)
        times[K] = best
        _STATE.pop("st", None)
    if saved is not None:
        _STATE["st"] = saved
    return (times[k1] - times[k0]) / (k1 - k0) * 1e9

